# revision 7
# baseline (speedup 1.0000x reference)
"""GCN-sampling (NodeFlow) kernel for 8 Trainium2 NeuronCores.

Strategy (single NEFF, SPMD by data, no collectives):
  - features table padded to [N0, 512] f16, replicated to all cores.
  - Layer-1 nodes (N1=25000) sharded 8-way (3125/core).
  - Stage 1 per core: dma_gather raw feature rows for its nodes' neighbors
    (index lists sorted by node-superblock x 32768-row window so int16
    indices fit), aggregate the 16-neighbor mean via one-hot selection
    matmuls (S_b built on-device with is_equal vs an iota row), accumulate
    node-major m0 in PSUM, then W1 matmul + bias + relu + concat -> Q rows
    (Q = h1cat @ W2/16) written to a per-core DRAM table.
  - Stage 2 per core: dma_gather local Q rows for seed neighbors owned by
    this core and partial-sum over all 5000 seeds with the same selection
    trick. Host sums the 8 partials and adds b2 (unsharding).
All matmuls f16 x f16 -> f32 PSUM. 1/16 mean factors folded into W1/W2.
"""

import sys

sys.path.insert(0, "/opt/trn_rl_repo")

import numpy as np

import concourse.bass as bass
import concourse.mybir as mybir
from concourse import bacc
from concourse.tile import TileContext
from concourse.masks import make_identity
from concourse.bass_utils import run_bass_kernel_spmd

N0, N1, N2 = 200000, 25000, 5000
FANOUT = 16
IN_F, NH, NCLS = 500, 128, 47
NCORES = 8
WINDOW = 32768
NWIN = (N0 + WINDOW - 1) // WINDOW  # 7
E1 = 512  # padded feature row (f16 -> 1024B)
E2 = 128  # padded Q row (f16 -> 256B)
NODES_PER_CORE = N1 // NCORES  # 3125
NSB1 = (NODES_PER_CORE + 127) // 128  # 25 node superblocks per core
SEEDS = N2
NSB2 = (SEEDS + 127) // 128  # 40 seed superblocks
GROUP1 = 5  # sb1 per gather chunk group (psum banks)
GROUP2 = 5
QROWS = NSB1 * 128  # 3200 rows in per-core Q table

f16 = mybir.dt.float16
f32 = mybir.dt.float32
i16 = mybir.dt.int16


def _wrap_idxs(flat):
    """[n] -> [128, n/16] int16: index i at [i%16, i//16], replicated x8."""
    n = len(flat)
    assert n % 16 == 0
    a = np.empty((128, n // 16), np.int16)
    blk = flat.reshape(n // 16, 16).T
    for g in range(8):
        a[g * 16 : (g + 1) * 16, :] = blk
    return a


def _plan_stage1(src0):
    """Per-core per-(sb, window) index/nid lists with uniform capacities.

    Returns caps [NSB1][NWIN] and per-core (idx16, nid) cell arrays.
    """
    counts = np.zeros((NCORES, NSB1, NWIN), np.int64)
    cells = [[[None] * NWIN for _ in range(NSB1)] for _ in range(NCORES)]
    for c in range(NCORES):
        s = src0[c * NODES_PER_CORE : (c + 1) * NODES_PER_CORE]  # [3125, 16]
        nloc = np.repeat(np.arange(s.shape[0]), FANOUT)
        flat = s.reshape(-1)
        w_of = flat // WINDOW
        sb_of = nloc // 128
        order = np.lexsort((flat, w_of, sb_of))
        flat, nloc, w_of, sb_of = flat[order], nloc[order], w_of[order], sb_of[order]
        key = sb_of * NWIN + w_of
        starts = np.searchsorted(key, np.arange(NSB1 * NWIN))
        ends = np.searchsorted(key, np.arange(NSB1 * NWIN), side="right")
        for sb in range(NSB1):
            for w in range(NWIN):
                k = sb * NWIN + w
                li = flat[starts[k] : ends[k]] - w * WINDOW
                ln = nloc[starts[k] : ends[k]] - sb * 128
                counts[c, sb, w] = len(li)
                cells[c][sb][w] = (li, ln)
    caps = ((counts.max(axis=0) + 127) // 128) * 128  # [NSB1, NWIN]
    # pack per-core cell arrays to capacity, keyed by (sb, w) for
    # chunk-order emission later
    packed = []
    for c in range(NCORES):
        pc = {}
        for sb in range(NSB1):
            for w in range(NWIN):
                cap = caps[sb, w]
                if cap == 0:
                    continue
                li, ln = cells[c][sb][w]
                idx = np.zeros(cap, np.int16)
                nid = np.full(cap, -1.0, np.float16)
                idx[: len(li)] = li.astype(np.int16)
                nid[: len(ln)] = ln.astype(np.float16)
                if len(li) < cap:
                    idx[len(li) :] = li[-1] if len(li) else 0
                pc[(sb, w)] = (idx, nid)
        packed.append(pc)
    return caps, packed


def _plan_stage2(src1, node_perm_of=None):
    """Per-core per-sb2 lists of (local Q row, seed slot)."""
    counts = np.zeros((NCORES, NSB2), np.int64)
    cells = [[None] * NSB2 for _ in range(NCORES)]
    flat = src1.reshape(-1)  # values in [0, N1)
    seed = np.repeat(np.arange(SEEDS), FANOUT)
    owner = flat // NODES_PER_CORE
    local = flat % NODES_PER_CORE
    sb2 = seed // 128
    for c in range(NCORES):
        m = owner == c
        lc, sc, s2 = local[m], seed[m], sb2[m]
        order = np.lexsort((lc, s2))
        lc, sc, s2 = lc[order], sc[order], s2[order]
        starts = np.searchsorted(s2, np.arange(NSB2))
        ends = np.searchsorted(s2, np.arange(NSB2), side="right")
        for k in range(NSB2):
            li = lc[starts[k] : ends[k]]
            ln = sc[starts[k] : ends[k]] - k * 128
            counts[c, k] = len(li)
            cells[c][k] = (li, ln)
    caps = ((counts.max(axis=0) + 127) // 128) * 128  # [NSB2]
    packed = []
    for c in range(NCORES):
        pc = []
        for k in range(NSB2):
            cap = caps[k]
            if cap == 0:
                continue
            li, ln = cells[c][k]
            idx = np.zeros(cap, np.int16)
            nid = np.full(cap, -1.0, np.float16)
            idx[: len(li)] = li.astype(np.int16)
            nid[: len(ln)] = ln.astype(np.float16)
            if len(li) < cap:
                idx[len(li) :] = li[-1] if len(li) else 0
            pc.append((k, idx, nid))
        packed.append(pc)
    return caps, packed


def _build_chunks(caps1):
    """Stage-1 chunk schedule: for each sb-group, for each window, one
    dma_gather covering the group's cells in that window. Returns a list of
    (w, [(sb, cap), ...]) in emission order, plus per-sb block sequencing."""
    chunks = []
    for g0 in range(0, NSB1, GROUP1):
        sbs = list(range(g0, min(g0 + GROUP1, NSB1)))
        for w in range(NWIN):
            cells = [(sb, int(caps1[sb, w])) for sb in sbs if caps1[sb, w] > 0]
            if cells:
                chunks.append((w, cells))
    return chunks


def build_kernel(caps1, caps2):
    nc = bacc.Bacc(None, target_bir_lowering=False, debug=False)

    chunks1 = _build_chunks(caps1)
    tot1 = int(sum(cap for _, cells in chunks1 for _, cap in cells))
    nb1 = tot1 // 128
    s2cells = [(k, int(caps2[k])) for k in range(NSB2) if caps2[k] > 0]
    tot2 = int(sum(cap for _, cap in s2cells))
    nb2 = tot2 // 128

    ftab = nc.dram_tensor("ftab", [N0, E1], f16, kind="ExternalInput")
    idx1 = nc.dram_tensor("idx1", [128, tot1 // 16], i16, kind="ExternalInput")
    nid1 = nc.dram_tensor("nid1", [128, nb1], f16, kind="ExternalInput")
    idx2 = nc.dram_tensor("idx2", [128, tot2 // 16], i16, kind="ExternalInput")
    nid2 = nc.dram_tensor("nid2", [128, nb2], f16, kind="ExternalInput")
    w1t = nc.dram_tensor("w1t", [128, 4, NH], f16, kind="ExternalInput")  # W1/16 chunks
    b1v = nc.dram_tensor("b1v", [128, 1], f32, kind="ExternalInput")
    w2t = nc.dram_tensor("w2t", [128, 2, NCLS], f16, kind="ExternalInput")  # W2/16
    iot = nc.dram_tensor("iot", [128, 128], f16, kind="ExternalInput")
    partial = nc.dram_tensor("partial", [NSB2 * 128, NCLS], f32, kind="ExternalOutput")

    # per-sb global block sequence lengths for start/stop flags
    sb_nblocks = {sb: int(sum(caps1[sb, w] for w in range(NWIN))) // 128 for sb in range(NSB1)}

    with TileContext(nc) as tc:
        with (
            tc.tile_pool(name="const", bufs=1) as cpool,
            tc.tile_pool(name="gather", bufs=3) as gpool,
            tc.tile_pool(name="sel", bufs=4) as spool,
            tc.tile_pool(name="epi", bufs=3) as epool,
            tc.tile_pool(name="m0psum", bufs=GROUP1, space="PSUM") as mpool,
            tc.tile_pool(name="epipsum", bufs=3, space="PSUM") as eppool,
            tc.tile_pool(name="dram", bufs=1, space="DRAM") as dpool,
        ):
            idx1_t = cpool.tile([128, tot1 // 16], i16)
            nc.sync.dma_start(idx1_t[:], idx1[:])
            nid1_t = cpool.tile([128, nb1], f16)
            nc.sync.dma_start(nid1_t[:], nid1[:])
            idx2_t = cpool.tile([128, tot2 // 16], i16)
            nc.sync.dma_start(idx2_t[:], idx2[:])
            nid2_t = cpool.tile([128, nb2], f16)
            nc.sync.dma_start(nid2_t[:], nid2[:])
            w1_t = cpool.tile([128, 4, NH], f16)
            nc.sync.dma_start(w1_t[:], w1t[:])
            b1_t = cpool.tile([128, 1], f32)
            nc.sync.dma_start(b1_t[:], b1v[:])
            w2_t = cpool.tile([128, 2, NCLS], f16)
            nc.sync.dma_start(w2_t[:], w2t[:])
            iota_t = cpool.tile([128, 128], f16)
            nc.sync.dma_start(iota_t[:], iot[:])
            ident = cpool.tile([128, 128], f16)
            make_identity(nc, ident[:])

            qtab = dpool.tile([QROWS, E2], f16)

            m0_psum = {}  # sb -> psum tile
            sb_seq = {sb: 0 for sb in range(NSB1)}  # blocks consumed per sb
            col = 0  # idx1 column cursor (int16 cols)
            blk = 0  # global block cursor (nid1 col)

            def epilogue(sb):
                p = m0_psum.pop(sb)
                m0_s = epool.tile([128, E1], f16, tag="m0s")
                nc.scalar.activation(m0_s[:], p[:], mybir.ActivationFunctionType.Copy)
                # transpose m0 [128 nodes, 512] -> 4x [128 feat, 128 nodes]
                h1p = eppool.tile([128, 128], f32, tag="ep")
                for k in range(4):
                    tp = eppool.tile([128, 128], f16, tag="ep")
                    nc.tensor.transpose(tp[:], m0_s[:, k * 128 : (k + 1) * 128], ident[:])
                    mt = epool.tile([128, 128], f16, tag="mt")
                    nc.vector.tensor_copy(mt[:], tp[:])
                    nc.tensor.matmul(
                        out=h1p[:], lhsT=w1_t[:, k, :], rhs=mt[:],
                        start=(k == 0), stop=(k == 3),
                    )
                h1_s = epool.tile([128, 128], f16, tag="h1")
                r_s = epool.tile([128, 128], f16, tag="r")
                nc.scalar.activation(h1_s[:], h1p[:], mybir.ActivationFunctionType.Identity, bias=b1_t[:, :1])
                nc.scalar.activation(r_s[:], h1p[:], mybir.ActivationFunctionType.Relu, bias=b1_t[:, :1])
                qp = eppool.tile([47, 128], f32, tag="ep")
                nc.tensor.matmul(out=qp[:], lhsT=w2_t[:, 0, :], rhs=h1_s[:], start=True, stop=False)
                nc.tensor.matmul(out=qp[:], lhsT=w2_t[:, 1, :], rhs=r_s[:], start=False, stop=True)
                qT_s = epool.tile([47, 128], f16, tag="qT")
                nc.vector.tensor_copy(qT_s[:], qp[:])
                q2p = eppool.tile([128, 128], f16, tag="ep")
                nc.tensor.transpose(q2p[:, :47], qT_s[:], ident[:47, :47])
                q_s = epool.tile([128, E2], f16, tag="qs")
                nc.vector.tensor_copy(q_s[:, :47], q2p[:, :47])
                nc.vector.memset(q_s[:, 47:], 0.0)
                nc.sync.dma_start(qtab[sb * 128 : (sb + 1) * 128, :], q_s[:])

            MAXIDX = 1024  # hw limit per dma_gather instruction
            for w, cells in chunks1:
                n = sum(cap for _, cap in cells)
                g_t = gpool.tile([128, n // 128, E1], f16, tag="g1")
                wsz = min(WINDOW, N0 - w * WINDOW)
                off = 0
                while off < n:
                    m = min(MAXIDX, n - off)
                    nc.gpsimd.dma_gather(
                        out_ap=g_t[:, off // 128 : (off + m) // 128, :],
                        in_ap=ftab[w * WINDOW : w * WINDOW + wsz, :],
                        idxs_ap=idx1_t[:, col + off // 16 : col + (off + m) // 16],
                        num_idxs=m,
                        num_idxs_reg=m,
                        elem_size=E1,
                    )
                    off += m
                col += n // 16
                b = 0
                for sb, cap in cells:
                    if sb not in m0_psum:
                        m0_psum[sb] = mpool.tile([128, E1], f32, tag="m0", name=f"m0_{sb}")
                    for _ in range(cap // 128):
                        s_b = spool.tile([128, 128], f16, tag="sb")
                        nc.vector.tensor_tensor(
                            out=s_b[:],
                            in0=nid1_t[:, blk : blk + 1].to_broadcast([128, 128]),
                            in1=iota_t[:],
                            op=mybir.AluOpType.is_equal,
                        )
                        seq = sb_seq[sb]
                        nc.tensor.matmul(
                            out=m0_psum[sb][:],
                            lhsT=s_b[:],
                            rhs=g_t[:, b, :],
                            start=(seq == 0),
                            stop=(seq == sb_nblocks[sb] - 1),
                        )
                        sb_seq[sb] = seq + 1
                        b += 1
                        blk += 1
                    if sb_seq[sb] == sb_nblocks[sb]:
                        epilogue(sb)

            # ---- stage 2 ----
            col2 = 0
            blk2 = 0
            for g0 in range(0, len(s2cells), GROUP2):
                cells = s2cells[g0 : g0 + GROUP2]
                n = sum(cap for _, cap in cells)
                g_t = gpool.tile([128, n // 128, E2], f16, tag="g2")
                off = 0
                while off < n:
                    m = min(MAXIDX, n - off)
                    nc.gpsimd.dma_gather(
                        out_ap=g_t[:, off // 128 : (off + m) // 128, :],
                        in_ap=qtab[:],
                        idxs_ap=idx2_t[:, col2 + off // 16 : col2 + (off + m) // 16],
                        num_idxs=m,
                        num_idxs_reg=m,
                        elem_size=E2,
                    )
                    off += m
                col2 += n // 16
                b = 0
                for k, cap in cells:
                    pp = eppool.tile([128, 128], f32, tag="ep")
                    nblk = cap // 128
                    for j in range(nblk):
                        s_b = spool.tile([128, 128], f16, tag="sb")
                        nc.vector.tensor_tensor(
                            out=s_b[:],
                            in0=nid2_t[:, blk2 : blk2 + 1].to_broadcast([128, 128]),
                            in1=iota_t[:],
                            op=mybir.AluOpType.is_equal,
                        )
                        nc.tensor.matmul(
                            out=pp[:],
                            lhsT=s_b[:],
                            rhs=g_t[:, b, :],
                            start=(j == 0),
                            stop=(j == nblk - 1),
                        )
                        b += 1
                        blk2 += 1
                    p_s = epool.tile([128, NCLS], f32, tag="ps")
                    nc.vector.tensor_copy(p_s[:], pp[:, :NCLS])
                    nc.sync.dma_start(partial[k * 128 : (k + 1) * 128, :], p_s[:])
    nc.compile()
    return nc, chunks1, s2cells


def _host_inputs(features, src0, src1, W1, b1, W2):
    caps1, packed1 = _plan_stage1(np.asarray(src0))
    caps2, packed2 = _plan_stage2(np.asarray(src1))

    ftab_np = np.zeros((N0, E1), np.float16)
    ftab_np[:, :IN_F] = np.asarray(features, np.float32).astype(np.float16)

    w1_np = np.zeros((128, 4, NH), np.float16)
    w1f = np.zeros((E1, NH), np.float32)
    w1f[:IN_F] = np.asarray(W1, np.float32) / FANOUT
    for k in range(4):
        w1_np[:, k, :] = w1f[k * 128 : (k + 1) * 128].astype(np.float16)
    b1_np = np.asarray(b1, np.float32).reshape(128, 1)
    w2_np = np.zeros((128, 2, NCLS), np.float16)
    w2f = np.asarray(W2, np.float32) / FANOUT
    w2_np[:, 0, :] = w2f[:NH].astype(np.float16)
    w2_np[:, 1, :] = w2f[NH:].astype(np.float16)
    iota_np = np.tile(np.arange(128, dtype=np.float16), (128, 1))

    # emission order: chunks (group-major, window-inner), cells within chunk
    chunks1 = _build_chunks(caps1)
    cell_order = [(sb, w) for w, cells in chunks1 for sb, _ in cells]

    in_maps = []
    for c in range(NCORES):
        idxs = np.concatenate([_wrap_idxs(packed1[c][k][0]) for k in cell_order], axis=1)
        nids = np.concatenate(
            [packed1[c][k][1].reshape(-1, 128) for k in cell_order], axis=0
        ).T
        idxs2 = np.concatenate([_wrap_idxs(idx) for _, idx, _ in packed2[c]], axis=1)
        nids2 = np.concatenate([nid.reshape(-1, 128) for _, _, nid in packed2[c]], axis=0).T
        in_maps.append(
            {
                "ftab": ftab_np,
                "idx1": np.ascontiguousarray(idxs),
                "nid1": np.ascontiguousarray(nids.astype(np.float16)),
                "idx2": np.ascontiguousarray(idxs2),
                "nid2": np.ascontiguousarray(nids2.astype(np.float16)),
                "w1t": w1_np,
                "b1v": b1_np,
                "w2t": w2_np,
                "iot": iota_np,
            }
        )
    return caps1, caps2, in_maps


_cache = {}


def kernel(features, src0, src1, W1, b1, W2, b2):
    caps1, caps2, in_maps = _host_inputs(features, src0, src1, W1, b1, W2)
    key = (caps1.tobytes(), caps2.tobytes())
    if key not in _cache:
        _cache[key] = build_kernel(caps1, caps2)
    nc, _, _ = _cache[key]
    res = run_bass_kernel_spmd(nc, in_maps, core_ids=list(range(NCORES)))
    out = np.zeros((SEEDS, NCLS), np.float64)
    for c in range(NCORES):
        out += res.results[c]["partial"][:SEEDS].astype(np.float64)
    out = out + np.asarray(b2, np.float64)[None, :]
    return out.astype(np.float32)


if __name__ == "__main__":
    rng = np.random.default_rng(0)
    feats = rng.standard_normal((N0, IN_F), dtype=np.float32)
    src0 = rng.integers(0, N0, size=(N1, FANOUT))
    src1 = rng.integers(0, N1, size=(N2, FANOUT))
    W1 = rng.standard_normal((IN_F, NH), dtype=np.float32) * 0.05
    b1 = np.zeros(NH, np.float32)
    W2 = rng.standard_normal((2 * NH, NCLS), dtype=np.float32) * 0.05
    b2 = np.zeros(NCLS, np.float32)
    out = kernel(feats, src0, src1, W1, b1, W2, b2)
    m0 = feats[src0].mean(axis=1)
    h1 = m0 @ W1 + b1
    h1 = np.concatenate([h1, np.maximum(h1, 0)], axis=1)
    m1 = h1[src1].mean(axis=1)
    ref = m1 @ W2 + b2
    rel = np.abs(out - ref) / (np.abs(ref) + 1e-5)
    print("max rel err:", rel.max(), "mean:", rel.mean())
    print("norm rel:", np.linalg.norm(out - ref) / np.linalg.norm(ref))
